# revision 13
# baseline (speedup 1.0000x reference)
"""Trainium2 Bass kernel for nn_MaxROI (NMS-style ROI extraction). v3"""

import numpy as np

B, N = 256, 65536
NCORES = 8
RPC = B // NCORES            # samples per core: 32
K, MAX_NUM = 24, 5
NSEL = K + MAX_NUM           # 29
NCHUNK = 4                   # column chunks per row -> 32*4 = 128 partitions
CHUNK = N // NCHUNK          # 16384
COLT = 4096                  # d columns per streamed tile (per partition)
NT = CHUNK // COLT           # 4 streamed tiles
WIN = 2048                   # top-8 window (exactness verified offline)
WPT = COLT // WIN            # windows per tile: 2
NWIN = CHUNK // WIN          # windows per partition: 8
NCAND = NWIN * 8             # candidates per partition: 64
NMRG = NCHUNK * NCAND        # merged candidates per sample: 256
NEG = -1.0e30
BIGM = float(1 << 20)
NCONST = 2176


def build_consts() -> np.ndarray:
    c = np.zeros((128, NCONST), np.float32)
    p = np.arange(128)
    i = np.arange(NCAND)
    # global-in-row column base of candidate slot i on partition p
    c[:, 0:NCAND] = (i[None, :] // 8) * WIN + (p[:, None] % NCHUNK) * CHUNK
    # position index 0..255 tiled 8x (for the wide one-hot gather)
    c[:, 64:64 + 8 * NMRG] = np.tile(np.arange(NMRG), 8)[None, :]
    # flat box-table base for partition p: (p % 32) * N
    c[:, 2112] = (p // NCHUNK) * N
    c[0:RPC, 2113:2113 + K] = BIGM + np.arange(K)[None, :]
    return c


def _build_kernel():
    import os
    import concourse.bacc as bacc
    import concourse.bass as bass
    import concourse.tile as tile
    from concourse import mybir

    IND1 = int(os.environ.get("MAXROI_IND1", "0"))  # multi-offset indirect
    STAGE = int(os.environ.get("MAXROI_STAGE", "5"))

    f32 = mybir.dt.float32
    u16 = mybir.dt.uint16
    u32 = mybir.dt.uint32
    Op = mybir.AluOpType
    AX = mybir.AxisListType

    nc = bacc.Bacc("TRN2", target_bir_lowering=False, debug=False,
                   num_devices=NCORES)
    scores = nc.dram_tensor("scores", [RPC, N * 2], f32, kind="ExternalInput")
    boxesf = nc.dram_tensor("boxes", [RPC * N, 4], f32, kind="ExternalInput")
    consts = nc.dram_tensor("consts", [128, NCONST], f32, kind="ExternalInput")
    rois = nc.dram_tensor("rois", [RPC, MAX_NUM * 4], f32, kind="ExternalOutput")

    from concourse.tile import add_dep_helper

    with tile.TileContext(nc) as tc:
        with (
            tc.tile_pool(name="stream", bufs=3) as spool,
            tc.tile_pool(name="dbuf", bufs=2) as dpool,
            tc.tile_pool(name="persist", bufs=1) as pp,
            tc.tile_pool(name="small", bufs=1) as sp,
        ):
            cand = pp.tile([128, NCAND], f32)
            cidxu = pp.tile([128, NCAND], u16)
            ct = pp.tile([128, NCONST], f32)
            nc.gpsimd.dma_start(out=ct[:, :], in_=consts.ap())
            # DMA-permuted scratch (rearranged DMA APs are under-tracked by
            # Tile's dep tracker -> order users explicitly via add_dep_helper)
            rvals = pp.tile([RPC, NMRG], f32)
            ridx4 = pp.tile([128, NMRG], f32)
            rmpf8 = pp.tile([128, 8], f32)
            idxall = pp.tile([128, 8], u32)
            gbpall = pp.tile([128, 32], f32)
            gboxd = pp.tile([RPC, 32, 4], f32)

            # ---- stage 1: stream scores, d = s1 - s0, top-8 per window;
            #      per-tile: resolve columns and fold candidates per sample
            #      (hidden under the stream) ----
            sview = scores.ap().rearrange("r (c q) -> r c q", c=NCHUNK)
            cidxf = sp.tile([128, NCAND], f32)
            candv = cand[:, :].rearrange("(r c) i -> r c i", c=NCHUNK)
            cxv = cidxf[:, :].rearrange("(r c) i -> r c i", c=NCHUNK)
            r4v = ridx4[:, :].rearrange("(r g) i -> r g i", g=NCHUNK)
            fvs = []
            f0s = []
            cis = []
            TILES = [4096, 4096, 4096, 2048, 2048]
            colpos = 0
            for t, tcols in enumerate(TILES):
                st = spool.tile([128, COLT * 2], f32, tag="st")
                base = colpos * 2
                for c in range(NCHUNK):
                    eng = nc.sync if (t + c) % 2 == 0 else nc.scalar
                    eng.dma_start(
                        out=st[:, 0:tcols * 2].rearrange(
                            "(r c) q -> r c q", c=NCHUNK)[:, c, :],
                        in_=sview[:, c, base:base + 2 * tcols])
                d = dpool.tile([128, COLT], f32, tag="d")
                s3 = st[:, 0:tcols * 2].rearrange("p (q two) -> p q two",
                                                  two=2)
                nc.gpsimd.tensor_tensor(
                    out=d[:, 0:tcols], in0=s3[:, :, 1], in1=s3[:, :, 0],
                    op=Op.subtract,
                )
                o8 = (colpos // WIN) * 8
                wpt = tcols // WIN
                nw8 = wpt * 8
                for w in range(wpt):
                    ow = o8 + w * 8
                    seg = d[:, w * WIN:(w + 1) * WIN]
                    nc.vector.max(out=cand[:, ow:ow + 8], in_=seg)
                    nc.vector.max_index(
                        out=cidxu[:, ow:ow + 8],
                        in_max=cand[:, ow:ow + 8],
                        in_values=seg,
                    )
                # columns for this tile's candidates
                nc.vector.tensor_copy(out=cidxf[:, o8:o8 + nw8],
                                      in_=cidxu[:, o8:o8 + nw8])
                ci = nc.vector.tensor_tensor(
                    out=cidxf[:, o8:o8 + nw8], in0=cidxf[:, o8:o8 + nw8],
                    in1=ct[:, o8:o8 + nw8], op=Op.add)
                cis.append(ci)
                colpos += tcols
            for c in range(NCHUNK):
                eng = nc.sync if c % 2 == 0 else nc.scalar
                fv = eng.dma_start(
                    out=rvals[:, NCAND * c:NCAND * (c + 1)],
                    in_=candv[:, c, :])
                for ci in cis:
                    add_dep_helper(fv.ins, ci.ins, reason="cand ready")
                fvs.append(fv)
                fi = eng.dma_start(
                    out=r4v[:, 0, NCAND * c:NCAND * (c + 1)],
                    in_=cxv[:, c, :])
                for ci in cis:
                    add_dep_helper(fi.ins, ci.ins, reason="cidxf ready")
                f0s.append(fi)
            fis = list(f0s)
            for g in range(1, NCHUNK):
                eng = nc.sync if g % 2 == 0 else nc.scalar
                fi = eng.dma_start(out=r4v[:, g, :], in_=r4v[:, 0, :])
                for f0 in f0s:
                    add_dep_helper(fi.ins, f0.ins, reason="ridx g0 ready")
                fis.append(fi)

            # ---- top-32 per sample over the 256 merged candidates ----
            rm8 = sp.tile([RPC, 32], f32)
            rmpu = sp.tile([RPC, 32], u16)
            for g in range(4):
                v8 = rm8[:, g * 8:g * 8 + 8]
                mi = nc.vector.max(out=v8, in_=rvals[:, :])
                if g == 0:
                    for fv in fvs:
                        add_dep_helper(mi.ins, fv.ins, reason="rvals ready")
                nc.vector.max_index(out=rmpu[:, g * 8:g * 8 + 8],
                                    in_max=v8, in_values=rvals[:, :])
                if g < 3:
                    nc.vector.match_replace(
                        out=rvals[:, :], in_to_replace=v8,
                        in_values=rvals[:, :], imm_value=NEG)
            rmpf = sp.tile([RPC, 32], f32)
            rc = nc.vector.tensor_copy(out=rmpf[:, :], in_=rmpu[:, :])
            # spread winner positions to [128, 8] (partition 32g+r = winner
            # group g of sample r)
            fss = []
            r8v = rmpf8[:, :].rearrange("(r g) t -> r g t", g=NCHUNK)
            for g in range(NCHUNK):
                eng = nc.sync if g % 2 == 0 else nc.scalar
                fs = eng.dma_start(
                    out=r8v[:, g, :],
                    in_=rmpf[:, 8 * g:8 * (g + 1)])
                add_dep_helper(fs.ins, rc.ins, reason="rmpf ready")
                fss.append(fs)

            if STAGE < 3:
                zro = sp.tile([RPC, MAX_NUM * 4], f32)
                nc.vector.memset(zro[:, :], 0.0)
                zz = nc.vector.tensor_tensor(out=zro[:, 0:1], in0=rmpf[:, 0:1],
                                             in1=rmpf8.ap()[0:RPC, 0:1], op=Op.mult)
                for fs in fss:
                    add_dep_helper(zz.ins, fs.ins, reason="dbg")
                nc.sync.dma_start(out=rois.ap(), in_=zro[:, :])
                nc.compile()
                return nc

            # ---- wide one-hot gather: winner columns -> flat box indices,
            #      directly in the indirect-DMA index layout [128, 8] ----
            ohp = sp.tile([128, 8, NMRG], f32)
            idxf = sp.tile([128, 8], f32)
            iota8x = ct[:, 64:64 + 8 * NMRG].rearrange("p (t i) -> p t i", t=8)
            o1 = nc.vector.tensor_tensor(
                out=ohp[:, :, :], in0=iota8x,
                in1=rmpf8[:, :].unsqueeze(2).to_broadcast([128, 8, NMRG]),
                op=Op.is_equal)
            for fs in fss:
                add_dep_helper(o1.ins, fs.ins, reason="rmpf8 ready")
            o2 = nc.vector.tensor_tensor(
                out=ohp[:, :, :], in0=ohp[:, :, :],
                in1=ridx4[:, :].unsqueeze(1).to_broadcast([128, 8, NMRG]),
                op=Op.mult)
            for fi in fis:
                add_dep_helper(o2.ins, fi.ins, reason="ridx4 ready")
            nc.vector.tensor_reduce(out=idxf[:, :], in_=ohp[:, :, :],
                                    axis=AX.X, op=Op.add)
            nc.vector.tensor_scalar(idxf[:, :], idxf[:, :],
                                    ct[:, 2112:2113], None, op0=Op.add)
            gci = nc.vector.tensor_copy(out=idxall[:, :], in_=idxf[:, :])

            if STAGE < 4:
                zro = sp.tile([RPC, MAX_NUM * 4], f32)
                nc.vector.memset(zro[:, :], 0.0)
                zz = nc.vector.tensor_tensor(out=zro[:, 0:1], in0=idxf[:, 0:1][0:RPC, :],
                                             in1=idxf[0:RPC, 1:2], op=Op.mult)
                add_dep_helper(zz.ins, gci.ins, reason="dbg")
                nc.sync.dma_start(out=rois.ap(), in_=zro[:, :])
                nc.compile()
                return nc

            # ---- winner boxes via indirect DMA ----
            gis = []
            if IND1:
                gi = nc.gpsimd.indirect_dma_start(
                    out=gbpall[:, :], out_offset=None,
                    in_=boxesf.ap(),
                    in_offset=bass.IndirectOffsetOnAxis(
                        ap=idxall[:, 0:8], axis=0),
                )
                add_dep_helper(gi.ins, gci.ins, reason="idxall ready")
                gis.append(gi)
            else:
                for t in range(8):
                    gi = nc.gpsimd.indirect_dma_start(
                        out=gbpall[:, t * 4:(t + 1) * 4],
                        out_offset=None,
                        in_=boxesf.ap(),
                        in_offset=bass.IndirectOffsetOnAxis(
                            ap=idxall[:, t:t + 1], axis=0),
                    )
                    add_dep_helper(gi.ins, gci.ins, reason="idxall ready")
                    gis.append(gi)
            # regroup: gboxd[r, 8g+t, :] = gbpall[32g+r, 4t:4t+4]
            gb2 = gboxd.ap().rearrange("r k f -> r (k f)")
            rbs = []
            for g in range(NCHUNK):
                eng = nc.sync if g % 2 == 0 else nc.scalar
                rb = eng.dma_start(
                    out=gb2[:, 32 * g:32 * (g + 1)],
                    in_=gbpall.ap()[32 * g:32 * (g + 1), :])
                for gi in gis:
                    add_dep_helper(rb.ins, gi.ins, reason="gbp ready")
                rbs.append(rb)
            gbox = sp.tile([RPC, 32, 4], f32)
            cp = nc.vector.tensor_copy(
                out=gbox[:, :, :].rearrange("p a b -> p (a b)"),
                in_=gboxd[:, :, :].rearrange("r k f -> r (k f)"))
            for rb in rbs:
                add_dep_helper(cp.ins, rb.ins, reason="gboxd ready")

            if STAGE < 5:
                zro = sp.tile([RPC, MAX_NUM * 4], f32)
                nc.vector.memset(zro[:, :], 0.0)
                zz = nc.vector.tensor_tensor(out=zro[:, 0:4], in0=gbox[:, 0, :],
                                             in1=gbox[:, 1, :], op=Op.mult)
                nc.sync.dma_start(out=rois.ap(), in_=zro[:, :])
                nc.compile()
                return nc

            # ---- clustering ----
            iotab = ct[0:RPC, 2113:2113 + K]
            mask = sp.tile([RPC, K], f32)
            nc.vector.memset(mask[:, :], 1.0)
            roisb = sp.tile([RPC, MAX_NUM * 4], f32)

            keyed = sp.tile([RPC, K], f32)
            kmin = sp.tile([RPC, 1], f32)
            oh = sp.tile([RPC, K], f32)
            ohscr4 = sp.tile([RPC, K, 4], f32)
            mb = sp.tile([RPC, 4], f32)
            ixy1 = sp.tile([RPC, K, 2], f32)
            ixy2 = sp.tile([RPC, K, 2], f32)
            wh = sp.tile([RPC, K, 2], f32)
            inter = sp.tile([RPC, K], f32)
            awh = sp.tile([RPC, 2], f32)
            area_a = sp.tile([RPC, 1], f32)
            bwh = sp.tile([RPC, K, 2], f32)
            area_b = sp.tile([RPC, K], f32)
            union = sp.tile([RPC, K], f32)
            over = sp.tile([RPC, K], f32)
            nover = sp.tile([RPC, K], f32)
            tlo = sp.tile([RPC, K, 2], f32)
            thi = sp.tile([RPC, K, 2], f32)
            nxt = sp.tile([RPC, K], f32)
            s1 = sp.tile([RPC, 1], f32)
            e1 = sp.tile([RPC, 1], f32)
            e1u = sp.tile([RPC, 1], u32)
            b0wh = sp.tile([RPC, 2], f32)

            nc.vector.tensor_tensor(out=bwh[:, :, :],
                                    in0=gbox[:, 0:K, 2:4],
                                    in1=gbox[:, 0:K, 0:2],
                                    op=Op.subtract)
            nc.vector.tensor_tensor(out=area_b[:, :], in0=bwh[:, :, 0],
                                    in1=bwh[:, :, 1], op=Op.mult)

            for j in range(MAX_NUM - 1):
                nc.vector.scalar_tensor_tensor(
                    out=keyed[:, :], in0=mask[:, :], scalar=-BIGM,
                    in1=iotab, op0=Op.mult, op1=Op.add)
                nc.vector.tensor_reduce(out=kmin[:, :], in_=keyed[:, :],
                                        axis=AX.X, op=Op.min)
                nc.vector.tensor_tensor(
                    out=oh[:, :], in0=keyed[:, :],
                    in1=kmin[:, 0:1].to_broadcast([RPC, K]),
                    op=Op.is_equal)
                nc.vector.tensor_tensor(
                    out=ohscr4[:, :, :], in0=gbox[:, 0:K, :],
                    in1=oh[:, :].unsqueeze(2).to_broadcast([RPC, K, 4]),
                    op=Op.mult)
                nc.vector.tensor_reduce(
                    out=mb[:, :], in_=ohscr4[:, :, :].transpose([0, 2, 1]),
                    axis=AX.X, op=Op.add)
                nc.vector.tensor_tensor(
                    out=ixy1[:, :, :], in0=gbox[:, 0:K, 0:2],
                    in1=mb[:, 0:2].unsqueeze(1).to_broadcast([RPC, K, 2]),
                    op=Op.max)
                nc.vector.tensor_tensor(
                    out=ixy2[:, :, :], in0=gbox[:, 0:K, 2:4],
                    in1=mb[:, 2:4].unsqueeze(1).to_broadcast([RPC, K, 2]),
                    op=Op.min)
                nc.vector.tensor_tensor(out=wh[:, :, :], in0=ixy2[:, :, :],
                                        in1=ixy1[:, :, :], op=Op.subtract)
                nc.vector.tensor_scalar_max(wh[:, :, :], wh[:, :, :], 0.0)
                nc.vector.tensor_tensor(out=inter[:, :], in0=wh[:, :, 0],
                                        in1=wh[:, :, 1], op=Op.mult)
                nc.vector.tensor_tensor(out=awh[:, :], in0=mb[:, 2:4],
                                        in1=mb[:, 0:2], op=Op.subtract)
                nc.vector.tensor_tensor(out=area_a[:, :], in0=awh[:, 0:1],
                                        in1=awh[:, 1:2], op=Op.mult)
                nc.vector.scalar_tensor_tensor(
                    out=union[:, :], in0=area_b[:, :],
                    scalar=area_a[:, 0:1], in1=inter[:, :],
                    op0=Op.add, op1=Op.subtract)
                nc.vector.scalar_tensor_tensor(
                    out=over[:, :], in0=inter[:, :], scalar=2.0,
                    in1=union[:, :], op0=Op.mult, op1=Op.is_ge)
                nc.vector.tensor_tensor(out=over[:, :], in0=over[:, :],
                                        in1=mask[:, :], op=Op.mult)
                nc.vector.tensor_scalar(nover[:, :], over[:, :],
                                        -1.0, 1.0, op0=Op.mult, op1=Op.add)
                nc.vector.scalar_tensor_tensor(
                    out=tlo[:, :, :],
                    in0=nover[:, :].unsqueeze(2).to_broadcast([RPC, K, 2]),
                    scalar=1.0e30, in1=gbox[:, 0:K, 0:2],
                    op0=Op.mult, op1=Op.add)
                nc.vector.tensor_reduce(
                    out=roisb[:, j * 4:j * 4 + 2],
                    in_=tlo[:, :, :].transpose([0, 2, 1]),
                    axis=AX.X, op=Op.min)
                nc.vector.scalar_tensor_tensor(
                    out=thi[:, :, :],
                    in0=nover[:, :].unsqueeze(2).to_broadcast([RPC, K, 2]),
                    scalar=-1.0e30, in1=gbox[:, 0:K, 2:4],
                    op0=Op.mult, op1=Op.add)
                nc.vector.tensor_reduce(
                    out=roisb[:, j * 4 + 2:j * 4 + 4],
                    in_=thi[:, :, :].transpose([0, 2, 1]),
                    axis=AX.X, op=Op.max)
                if j < MAX_NUM - 2:
                    nc.vector.tensor_tensor(out=nxt[:, :], in0=mask[:, :],
                                            in1=over[:, :],
                                            op=Op.subtract)
                    nc.vector.tensor_reduce(out=s1[:, :], in_=nxt[:, :],
                                            axis=AX.X, op=Op.max)
                    nc.vector.tensor_scalar(e1[:, :], s1[:, :],
                                            -1.0, 1.0,
                                            op0=Op.mult, op1=Op.add)
                    nc.vector.tensor_scalar(mask[:, :], nxt[:, :],
                                            s1[:, 0:1], None, op0=Op.mult)
                    nc.vector.tensor_tensor(out=mask[:, 0:1],
                                            in0=mask[:, 0:1],
                                            in1=e1[:, 0:1], op=Op.add)
                    nc.vector.tensor_copy(out=e1u[:, :], in_=e1[:, :])
                    nc.vector.copy_predicated(
                        out=gbox[:, 0, :],
                        mask=e1u[:, 0:1].to_broadcast([RPC, 4]),
                        data=gbox[:, K + j, :])
                    nc.vector.tensor_tensor(out=b0wh[:, :],
                                            in0=gbox[:, 0, 2:4],
                                            in1=gbox[:, 0, 0:2],
                                            op=Op.subtract)
                    nc.vector.tensor_tensor(out=area_b[:, 0:1],
                                            in0=b0wh[:, 0:1],
                                            in1=b0wh[:, 1:2], op=Op.mult)
            nc.vector.tensor_copy(out=roisb[:, 16:20],
                                  in_=gbox[:, K + MAX_NUM - 2, :])
            nc.sync.dma_start(out=rois.ap(), in_=roisb[:, :])

    nc.compile()
    return nc


_NC = None


def _get_nc():
    global _NC
    if _NC is None:
        _NC = _build_kernel()
    return _NC


def kernel(boxes: np.ndarray, scores: np.ndarray) -> np.ndarray:
    from concourse.bass_utils import run_bass_kernel_spmd

    nc = _get_nc()
    cst = build_consts()
    in_maps = []
    for i in range(NCORES):
        rs = slice(i * RPC, (i + 1) * RPC)
        in_maps.append({
            "scores": np.ascontiguousarray(
                scores[rs].reshape(RPC, N * 2), dtype=np.float32),
            "boxes": np.ascontiguousarray(
                boxes[rs].reshape(RPC * N, 4), dtype=np.float32),
            "consts": cst,
        })
    res = run_bass_kernel_spmd(nc, in_maps, list(range(NCORES)))
    out = np.concatenate(
        [res.results[i]["rois"].reshape(RPC, MAX_NUM, 4)
         for i in range(NCORES)], axis=0)
    return out


# revision 14
# speedup vs baseline: 1.1822x; 1.1822x over previous
"""Trainium2 Bass kernel for nn_MaxROI (NMS-style ROI extraction). v3"""

import numpy as np

B, N = 256, 65536
NCORES = 8
RPC = B // NCORES            # samples per core: 32
K, MAX_NUM = 24, 5
NSEL = K + MAX_NUM           # 29
NCHUNK = 4                   # column chunks per row -> 32*4 = 128 partitions
CHUNK = N // NCHUNK          # 16384
COLT = 4096                  # d columns per streamed tile (per partition)
NT = CHUNK // COLT           # 4 streamed tiles
WIN = 2048                   # top-8 window (exactness verified offline)
WPT = COLT // WIN            # windows per tile: 2
NWIN = CHUNK // WIN          # windows per partition: 8
NCAND = NWIN * 8             # candidates per partition: 64
NMRG = NCHUNK * NCAND        # merged candidates per sample: 256
NEG = -1.0e30
BIGM = float(1 << 20)
NCONST = 2176


def build_consts() -> np.ndarray:
    c = np.zeros((128, NCONST), np.float32)
    p = np.arange(128)
    i = np.arange(NCAND)
    # global-in-row column base of candidate slot i on partition p
    c[:, 0:NCAND] = (i[None, :] // 8) * WIN + (p[:, None] % NCHUNK) * CHUNK
    # position index 0..255 tiled 8x (for the wide one-hot gather)
    c[:, 64:64 + 8 * NMRG] = np.tile(np.arange(NMRG), 8)[None, :]
    # flat box-table base for partition p: (p % 32) * N
    c[:, 2112] = (p // NCHUNK) * N
    c[0:RPC, 2113:2113 + K] = BIGM + np.arange(K)[None, :]
    return c


def _build_kernel():
    import os
    import concourse.bacc as bacc
    import concourse.bass as bass
    import concourse.tile as tile
    from concourse import mybir

    IND1 = int(os.environ.get("MAXROI_IND1", "0"))  # multi-offset indirect
    STAGE = int(os.environ.get("MAXROI_STAGE", "5"))

    f32 = mybir.dt.float32
    u16 = mybir.dt.uint16
    u32 = mybir.dt.uint32
    Op = mybir.AluOpType
    AX = mybir.AxisListType

    nc = bacc.Bacc("TRN2", target_bir_lowering=False, debug=False,
                   num_devices=NCORES)
    scores = nc.dram_tensor("scores", [RPC, N * 2], f32, kind="ExternalInput")
    boxesf = nc.dram_tensor("boxes", [RPC * N, 4], f32, kind="ExternalInput")
    consts = nc.dram_tensor("consts", [128, NCONST], f32, kind="ExternalInput")
    rois = nc.dram_tensor("rois", [RPC, MAX_NUM * 4], f32, kind="ExternalOutput")

    from concourse.tile import add_dep_helper

    with tile.TileContext(nc) as tc:
        with (
            tc.tile_pool(name="stream", bufs=3) as spool,
            tc.tile_pool(name="dbuf", bufs=2) as dpool,
            tc.tile_pool(name="persist", bufs=1) as pp,
            tc.tile_pool(name="small", bufs=1) as sp,
        ):
            cand = pp.tile([128, NCAND], f32)
            cidxu = pp.tile([128, NCAND], u16)
            ct = pp.tile([128, NCONST], f32)
            nc.sync.dma_start(out=ct[:, :], in_=consts.ap())
            # DMA-permuted scratch (rearranged DMA APs are under-tracked by
            # Tile's dep tracker -> order users explicitly via add_dep_helper)
            rvals = pp.tile([RPC, NMRG], f32)
            ridx4 = pp.tile([128, NMRG], f32)
            rmpf8 = pp.tile([128, 8], f32)
            idxall = pp.tile([128, 8], u32)
            gbpall = pp.tile([128, 32], f32)
            gboxd = pp.tile([RPC, 32, 4], f32)

            # ---- stage 1: stream scores, d = s1 - s0, top-8 per window;
            #      per-tile: resolve columns and fold candidates per sample
            #      (hidden under the stream) ----
            sview = scores.ap().rearrange("r (c q) -> r c q", c=NCHUNK)
            cidxf = sp.tile([128, NCAND], f32)
            candv = cand[:, :].rearrange("(r c) i -> r c i", c=NCHUNK)
            cxv = cidxf[:, :].rearrange("(r c) i -> r c i", c=NCHUNK)
            r4v = ridx4[:, :].rearrange("(r g) i -> r g i", g=NCHUNK)
            fvs = []
            f0s = []
            cis = []
            candws = []
            for t in range(NT):
                st = spool.tile([128, COLT * 2], f32, tag="st")
                base = t * COLT * 2
                for c in range(NCHUNK):
                    eng = nc.sync if (t + c) % 2 == 0 else nc.scalar
                    eng.dma_start(
                        out=st[:, :].rearrange("(r c) q -> r c q",
                                               c=NCHUNK)[:, c, :],
                        in_=sview[:, c, base:base + 2 * COLT])
                d = dpool.tile([128, COLT], f32, tag="d")
                s3 = st[:, :].rearrange("p (q two) -> p q two", two=2)
                nc.gpsimd.tensor_tensor(
                    out=d[:, :], in0=s3[:, :, 1], in1=s3[:, :, 0],
                    op=Op.subtract,
                )
                for w in range(WPT):
                    ow = (t * WPT + w) * 8
                    seg = d[:, w * WIN:(w + 1) * WIN]
                    mi = nc.vector.max(out=cand[:, ow:ow + 8], in_=seg)
                    candws.append(mi)
                    nc.vector.max_index(
                        out=cidxu[:, ow:ow + 8],
                        in_max=cand[:, ow:ow + 8],
                        in_values=seg,
                    )

            # ---- candidate global-in-row columns ----
            nc.vector.tensor_copy(out=cidxf[:, :], in_=cidxu[:, :])
            ci = nc.vector.tensor_tensor(out=cidxf[:, :], in0=cidxf[:, :],
                                         in1=ct[:, 0:NCAND], op=Op.add)
            cis.append(ci)
            for c in range(NCHUNK):
                eng = nc.sync if c % 2 == 0 else nc.scalar
                fv = eng.dma_start(
                    out=rvals[:, NCAND * c:NCAND * (c + 1)],
                    in_=candv[:, c, :])
                for w in candws:
                    add_dep_helper(fv.ins, w.ins, reason="cand ready")
                fvs.append(fv)
                fi = eng.dma_start(
                    out=r4v[:, 0, NCAND * c:NCAND * (c + 1)],
                    in_=cxv[:, c, :])
                for ci in cis:
                    add_dep_helper(fi.ins, ci.ins, reason="cidxf ready")
                f0s.append(fi)
            fis = list(f0s)
            for g in range(1, NCHUNK):
                eng = nc.sync if g % 2 == 0 else nc.scalar
                fi = eng.dma_start(out=r4v[:, g, :], in_=r4v[:, 0, :])
                for f0 in f0s:
                    add_dep_helper(fi.ins, f0.ins, reason="ridx g0 ready")
                fis.append(fi)

            # ---- top-32 per sample over the 256 merged candidates ----
            rm8 = sp.tile([RPC, 32], f32)
            rmpu = sp.tile([RPC, 32], u16)
            for g in range(4):
                v8 = rm8[:, g * 8:g * 8 + 8]
                mi = nc.vector.max(out=v8, in_=rvals[:, :])
                if g == 0:
                    for fv in fvs:
                        add_dep_helper(mi.ins, fv.ins, reason="rvals ready")
                nc.vector.max_index(out=rmpu[:, g * 8:g * 8 + 8],
                                    in_max=v8, in_values=rvals[:, :])
                if g < 3:
                    nc.vector.match_replace(
                        out=rvals[:, :], in_to_replace=v8,
                        in_values=rvals[:, :], imm_value=NEG)
            rmpf = sp.tile([RPC, 32], f32)
            rc = nc.vector.tensor_copy(out=rmpf[:, :], in_=rmpu[:, :])
            # spread winner positions to [128, 8] (partition 32g+r = winner
            # group g of sample r)
            fss = []
            r8v = rmpf8[:, :].rearrange("(r g) t -> r g t", g=NCHUNK)
            for g in range(NCHUNK):
                eng = nc.sync if g % 2 == 0 else nc.scalar
                fs = eng.dma_start(
                    out=r8v[:, g, :],
                    in_=rmpf[:, 8 * g:8 * (g + 1)])
                add_dep_helper(fs.ins, rc.ins, reason="rmpf ready")
                fss.append(fs)

            if STAGE < 3:
                zro = sp.tile([RPC, MAX_NUM * 4], f32)
                nc.vector.memset(zro[:, :], 0.0)
                zz = nc.vector.tensor_tensor(out=zro[:, 0:1], in0=rmpf[:, 0:1],
                                             in1=rmpf8.ap()[0:RPC, 0:1], op=Op.mult)
                for fs in fss:
                    add_dep_helper(zz.ins, fs.ins, reason="dbg")
                nc.sync.dma_start(out=rois.ap(), in_=zro[:, :])
                nc.compile()
                return nc

            # ---- wide one-hot gather: winner columns -> flat box indices,
            #      directly in the indirect-DMA index layout [128, 8] ----
            ohp = sp.tile([128, 8, NMRG], f32)
            idxf = sp.tile([128, 8], f32)
            iota8x = ct[:, 64:64 + 8 * NMRG].rearrange("p (t i) -> p t i", t=8)
            o1 = nc.vector.tensor_tensor(
                out=ohp[:, :, :], in0=iota8x,
                in1=rmpf8[:, :].unsqueeze(2).to_broadcast([128, 8, NMRG]),
                op=Op.is_equal)
            for fs in fss:
                add_dep_helper(o1.ins, fs.ins, reason="rmpf8 ready")
            o2 = nc.vector.tensor_tensor(
                out=ohp[:, :, :], in0=ohp[:, :, :],
                in1=ridx4[:, :].unsqueeze(1).to_broadcast([128, 8, NMRG]),
                op=Op.mult)
            for fi in fis:
                add_dep_helper(o2.ins, fi.ins, reason="ridx4 ready")
            nc.vector.tensor_reduce(out=idxf[:, :], in_=ohp[:, :, :],
                                    axis=AX.X, op=Op.add)
            nc.vector.tensor_scalar(idxf[:, :], idxf[:, :],
                                    ct[:, 2112:2113], None, op0=Op.add)
            gci = nc.vector.tensor_copy(out=idxall[:, :], in_=idxf[:, :])

            if STAGE < 4:
                zro = sp.tile([RPC, MAX_NUM * 4], f32)
                nc.vector.memset(zro[:, :], 0.0)
                zz = nc.vector.tensor_tensor(out=zro[:, 0:1], in0=idxf[:, 0:1][0:RPC, :],
                                             in1=idxf[0:RPC, 1:2], op=Op.mult)
                add_dep_helper(zz.ins, gci.ins, reason="dbg")
                nc.sync.dma_start(out=rois.ap(), in_=zro[:, :])
                nc.compile()
                return nc

            # ---- winner boxes via indirect DMA ----
            gis = []
            if IND1:
                gi = nc.gpsimd.indirect_dma_start(
                    out=gbpall[:, :], out_offset=None,
                    in_=boxesf.ap(),
                    in_offset=bass.IndirectOffsetOnAxis(
                        ap=idxall[:, 0:8], axis=0),
                )
                add_dep_helper(gi.ins, gci.ins, reason="idxall ready")
                gis.append(gi)
            else:
                for t in range(8):
                    gi = nc.gpsimd.indirect_dma_start(
                        out=gbpall[:, t * 4:(t + 1) * 4],
                        out_offset=None,
                        in_=boxesf.ap(),
                        in_offset=bass.IndirectOffsetOnAxis(
                            ap=idxall[:, t:t + 1], axis=0),
                    )
                    add_dep_helper(gi.ins, gci.ins, reason="idxall ready")
                    gis.append(gi)
            # regroup: gboxd[r, 8g+t, :] = gbpall[32g+r, 4t:4t+4]
            gb2 = gboxd.ap().rearrange("r k f -> r (k f)")
            rbs = []
            for g in range(NCHUNK):
                eng = nc.sync if g % 2 == 0 else nc.scalar
                rb = eng.dma_start(
                    out=gb2[:, 32 * g:32 * (g + 1)],
                    in_=gbpall.ap()[32 * g:32 * (g + 1), :])
                for gi in gis:
                    add_dep_helper(rb.ins, gi.ins, reason="gbp ready")
                rbs.append(rb)
            gbox = sp.tile([RPC, 32, 4], f32)
            cp = nc.vector.tensor_copy(
                out=gbox[:, :, :].rearrange("p a b -> p (a b)"),
                in_=gboxd[:, :, :].rearrange("r k f -> r (k f)"))
            for rb in rbs:
                add_dep_helper(cp.ins, rb.ins, reason="gboxd ready")

            if STAGE < 5:
                zro = sp.tile([RPC, MAX_NUM * 4], f32)
                nc.vector.memset(zro[:, :], 0.0)
                zz = nc.vector.tensor_tensor(out=zro[:, 0:4], in0=gbox[:, 0, :],
                                             in1=gbox[:, 1, :], op=Op.mult)
                nc.sync.dma_start(out=rois.ap(), in_=zro[:, :])
                nc.compile()
                return nc

            # ---- clustering ----
            iotab = ct[0:RPC, 2113:2113 + K]
            mask = sp.tile([RPC, K], f32)
            nc.vector.memset(mask[:, :], 1.0)
            roisb = sp.tile([RPC, MAX_NUM * 4], f32)

            keyed = sp.tile([RPC, K], f32)
            kmin = sp.tile([RPC, 1], f32)
            oh = sp.tile([RPC, K], f32)
            ohscr4 = sp.tile([RPC, K, 4], f32)
            mb = sp.tile([RPC, 4], f32)
            ixy1 = sp.tile([RPC, K, 2], f32)
            ixy2 = sp.tile([RPC, K, 2], f32)
            wh = sp.tile([RPC, K, 2], f32)
            inter = sp.tile([RPC, K], f32)
            awh = sp.tile([RPC, 2], f32)
            area_a = sp.tile([RPC, 1], f32)
            bwh = sp.tile([RPC, K, 2], f32)
            area_b = sp.tile([RPC, K], f32)
            union = sp.tile([RPC, K], f32)
            over = sp.tile([RPC, K], f32)
            nover = sp.tile([RPC, K], f32)
            tlo = sp.tile([RPC, K, 2], f32)
            thi = sp.tile([RPC, K, 2], f32)
            nxt = sp.tile([RPC, K], f32)
            s1 = sp.tile([RPC, 1], f32)
            e1 = sp.tile([RPC, 1], f32)
            e1u = sp.tile([RPC, 1], u32)
            b0wh = sp.tile([RPC, 2], f32)

            nc.vector.tensor_tensor(out=bwh[:, :, :],
                                    in0=gbox[:, 0:K, 2:4],
                                    in1=gbox[:, 0:K, 0:2],
                                    op=Op.subtract)
            nc.vector.tensor_tensor(out=area_b[:, :], in0=bwh[:, :, 0],
                                    in1=bwh[:, :, 1], op=Op.mult)

            for j in range(MAX_NUM - 1):
                nc.vector.scalar_tensor_tensor(
                    out=keyed[:, :], in0=mask[:, :], scalar=-BIGM,
                    in1=iotab, op0=Op.mult, op1=Op.add)
                nc.vector.tensor_reduce(out=kmin[:, :], in_=keyed[:, :],
                                        axis=AX.X, op=Op.min)
                nc.vector.tensor_tensor(
                    out=oh[:, :], in0=keyed[:, :],
                    in1=kmin[:, 0:1].to_broadcast([RPC, K]),
                    op=Op.is_equal)
                nc.vector.tensor_tensor(
                    out=ohscr4[:, :, :], in0=gbox[:, 0:K, :],
                    in1=oh[:, :].unsqueeze(2).to_broadcast([RPC, K, 4]),
                    op=Op.mult)
                nc.vector.tensor_reduce(
                    out=mb[:, :], in_=ohscr4[:, :, :].transpose([0, 2, 1]),
                    axis=AX.X, op=Op.add)
                nc.vector.tensor_tensor(
                    out=ixy1[:, :, :], in0=gbox[:, 0:K, 0:2],
                    in1=mb[:, 0:2].unsqueeze(1).to_broadcast([RPC, K, 2]),
                    op=Op.max)
                nc.vector.tensor_tensor(
                    out=ixy2[:, :, :], in0=gbox[:, 0:K, 2:4],
                    in1=mb[:, 2:4].unsqueeze(1).to_broadcast([RPC, K, 2]),
                    op=Op.min)
                nc.vector.tensor_tensor(out=wh[:, :, :], in0=ixy2[:, :, :],
                                        in1=ixy1[:, :, :], op=Op.subtract)
                nc.vector.tensor_scalar_max(wh[:, :, :], wh[:, :, :], 0.0)
                nc.vector.tensor_tensor(out=inter[:, :], in0=wh[:, :, 0],
                                        in1=wh[:, :, 1], op=Op.mult)
                nc.vector.tensor_tensor(out=awh[:, :], in0=mb[:, 2:4],
                                        in1=mb[:, 0:2], op=Op.subtract)
                nc.vector.tensor_tensor(out=area_a[:, :], in0=awh[:, 0:1],
                                        in1=awh[:, 1:2], op=Op.mult)
                nc.vector.scalar_tensor_tensor(
                    out=union[:, :], in0=area_b[:, :],
                    scalar=area_a[:, 0:1], in1=inter[:, :],
                    op0=Op.add, op1=Op.subtract)
                nc.vector.scalar_tensor_tensor(
                    out=over[:, :], in0=inter[:, :], scalar=2.0,
                    in1=union[:, :], op0=Op.mult, op1=Op.is_ge)
                nc.vector.tensor_tensor(out=over[:, :], in0=over[:, :],
                                        in1=mask[:, :], op=Op.mult)
                nc.vector.tensor_scalar(nover[:, :], over[:, :],
                                        -1.0, 1.0, op0=Op.mult, op1=Op.add)
                nc.vector.scalar_tensor_tensor(
                    out=tlo[:, :, :],
                    in0=nover[:, :].unsqueeze(2).to_broadcast([RPC, K, 2]),
                    scalar=1.0e30, in1=gbox[:, 0:K, 0:2],
                    op0=Op.mult, op1=Op.add)
                nc.vector.tensor_reduce(
                    out=roisb[:, j * 4:j * 4 + 2],
                    in_=tlo[:, :, :].transpose([0, 2, 1]),
                    axis=AX.X, op=Op.min)
                nc.vector.scalar_tensor_tensor(
                    out=thi[:, :, :],
                    in0=nover[:, :].unsqueeze(2).to_broadcast([RPC, K, 2]),
                    scalar=-1.0e30, in1=gbox[:, 0:K, 2:4],
                    op0=Op.mult, op1=Op.add)
                nc.vector.tensor_reduce(
                    out=roisb[:, j * 4 + 2:j * 4 + 4],
                    in_=thi[:, :, :].transpose([0, 2, 1]),
                    axis=AX.X, op=Op.max)
                if j < MAX_NUM - 2:
                    nc.vector.tensor_tensor(out=nxt[:, :], in0=mask[:, :],
                                            in1=over[:, :],
                                            op=Op.subtract)
                    nc.vector.tensor_reduce(out=s1[:, :], in_=nxt[:, :],
                                            axis=AX.X, op=Op.max)
                    nc.vector.tensor_scalar(e1[:, :], s1[:, :],
                                            -1.0, 1.0,
                                            op0=Op.mult, op1=Op.add)
                    nc.vector.tensor_scalar(mask[:, :], nxt[:, :],
                                            s1[:, 0:1], None, op0=Op.mult)
                    nc.vector.tensor_tensor(out=mask[:, 0:1],
                                            in0=mask[:, 0:1],
                                            in1=e1[:, 0:1], op=Op.add)
                    nc.vector.tensor_copy(out=e1u[:, :], in_=e1[:, :])
                    nc.vector.copy_predicated(
                        out=gbox[:, 0, :],
                        mask=e1u[:, 0:1].to_broadcast([RPC, 4]),
                        data=gbox[:, K + j, :])
                    nc.vector.tensor_tensor(out=b0wh[:, :],
                                            in0=gbox[:, 0, 2:4],
                                            in1=gbox[:, 0, 0:2],
                                            op=Op.subtract)
                    nc.vector.tensor_tensor(out=area_b[:, 0:1],
                                            in0=b0wh[:, 0:1],
                                            in1=b0wh[:, 1:2], op=Op.mult)
            nc.vector.tensor_copy(out=roisb[:, 16:20],
                                  in_=gbox[:, K + MAX_NUM - 2, :])
            nc.sync.dma_start(out=rois.ap(), in_=roisb[:, :])

    nc.compile()
    return nc


_NC = None


def _get_nc():
    global _NC
    if _NC is None:
        _NC = _build_kernel()
    return _NC


def kernel(boxes: np.ndarray, scores: np.ndarray) -> np.ndarray:
    from concourse.bass_utils import run_bass_kernel_spmd

    nc = _get_nc()
    cst = build_consts()
    in_maps = []
    for i in range(NCORES):
        rs = slice(i * RPC, (i + 1) * RPC)
        in_maps.append({
            "scores": np.ascontiguousarray(
                scores[rs].reshape(RPC, N * 2), dtype=np.float32),
            "boxes": np.ascontiguousarray(
                boxes[rs].reshape(RPC * N, 4), dtype=np.float32),
            "consts": cst,
        })
    res = run_bass_kernel_spmd(nc, in_maps, list(range(NCORES)))
    out = np.concatenate(
        [res.results[i]["rois"].reshape(RPC, MAX_NUM, 4)
         for i in range(NCORES)], axis=0)
    return out
